# revision 64
# baseline (speedup 1.0000x reference)
"""Multi-head attention ('general' similarity, softmax, out-proj) on 8 trn2
NeuronCores via Bass/Tile.

Sharding (head-split): core c handles batch b=c//2 and head group hp=c%2
(heads 4hp..4hp+4 = 2 pairs), over the FULL query range.  Each core projects
K/V only for its 4 heads (halving the projection work vs a q-split), computes
its heads' attention for all 2048 queries, and emits a PARTIAL output
projection; the host sums the two partials per batch.

Device layout trick: everything is kept feature-major ("transposed") so every
matmul contraction runs along SBUF partitions:
  Q_l^T[e,q]   = sum_d W_Q[d,e] Q^T[d,q]          (host supplies Q^T)
  K_hw^T[e,s]  = sum_d Wg[d,e] K_l^T[d,s]         (W_gen_S folded into W_K)
  scores^T[k,q]= sum_e K_hw^T[e,k] Q_l^T[e,q]     (row-packed: 2 heads share PE)
  P^T          = exp(scores^T)                     (head a: ScalarE exp; head b:
                 one DVE tensor_scalar emitting bf16 exp bits into int16)
  headaug^T    = sum_k Vaug[k,(v,1)] P^T[k,q]     (65th 'ones' col => rowsums free)
  out[q,o]    += sum_hv (head^T/rowsum)[hv,q] W_m[hv,o]   (partial, host-summed)

The 64 pipeline steps are indexed by g=(pair,qhalf) in 0..3 and kblock t in
0..15 -- the same shape as the previous q-split kernel, so the software
pipeline (LAG, part1/part2 positions) carries over unchanged.
"""
import sys
import types

import numpy as np
import ml_dtypes

# ---------------------------------------------------------------- axon shim --
# antenv in this image lacks axon_hooks; register the NTFF profiling hook
# ourselves so trace=True works when the caller asks for it.
def _ensure_axon_hooks():
    if 'antenv.axon_hooks' in sys.modules:
        return
    try:
        from trn_agent_boot.trn_boot import _ntff_profile_via_ctypes
        hook = _ntff_profile_via_ctypes('/opt/axon/libaxon_pjrt.so')
    except Exception:
        hook = None
    mod = types.ModuleType('antenv.axon_hooks')
    mod.get_axon_ntff_profile_hook = lambda: hook
    mod.set_axon_ntff_profile_hook = lambda h: None
    sys.modules['antenv.axon_hooks'] = mod


_ensure_axon_hooks()

import concourse.bass as bass
import concourse.mybir as mybir
import concourse.tile as tile
from concourse.bass_utils import run_bass_kernel_spmd

BF16 = mybir.dt.bfloat16
F32 = mybir.dt.float32
I16 = mybir.dt.int16

# one-instruction exp on DVE: bf16 bits of exp(s) ~= int16(round(s*128/ln2
# + (127*128 - 5.5))).  Rel err ~N(+1%, 1.8%); the +1% common-mode bias
# cancels in softmax normalization.  Verified bit-exact vs hardware model.
_EXPC1 = 128.0 / 0.6931471805599453
_EXPC2 = 16256.0 - 5.5

P = 128
D = 512          # model dim (= D_K = D_V = D_OUT)
H = 8            # total heads (host-side fold uses this)
SQF = 2048       # full query range per core
SQ = 1024        # query rows per pipeline group (half range)
SK = 2048        # key rows (full sequence)
HL = 4           # local heads per core
DH = 64
DL = HL * DH     # local feature width (256)
NJ = HL // 2     # local head pairs (2)
NG = 4           # pipeline groups: (pair, qhalf)
NKB = SK // P    # 16 key blocks
NQB = SQF // P   # 16 query blocks (full range)
ND = D // P      # 4 feature blocks
NDL = DL // P    # 2 local-feature blocks
EXPF = mybir.ActivationFunctionType.Exp
MULT = mybir.AluOpType.mult


# ------------------------------------------------------- walrus workaround --
# This container's walrus accepts only ONE embedded sync-wait per hw
# instruction. Move all but the last wait of any instruction onto single-wait
# NoOps inserted just before it in the same engine stream.
_SPLIT_CTR = [0]


def _split_multi_waits(nc, max_waits=1):
    def mk_nop(engine, wait):
        _SPLIT_CTR[0] += 1
        nop = mybir.InstNoOp(name=f"antsplitw-{_SPLIT_CTR[0]}", ins=[], outs=[])
        nop.engine = engine
        nop.sync_info = mybir.SyncInfo(on_wait=[wait], on_update=[])
        return nop

    for fn in nc.m.functions:
        for bb in fn.blocks:
            out = []
            changed = False
            for inst in bb.instructions:
                si = inst.sync_info
                waits = list(si.on_wait) if si is not None and si.on_wait else []
                if len(waits) > max_waits:
                    for w in waits[:-max_waits]:
                        out.append(mk_nop(inst.engine, w))
                    si.on_wait = waits[-max_waits:]
                    changed = True
                out.append(inst)
            if changed:
                bb.instructions = out


# ------------------------------------------------------------ device kernel --
def _build_nc():
    nc = bass.Bass("TRN2", target_bir_lowering=False, debug=False)

    qt_d = nc.declare_dram_parameter("qt", [D, SQF], BF16, isOutput=False)
    kt_d = nc.declare_dram_parameter("kt", [D, SK], BF16, isOutput=False)
    vt_d = nc.declare_dram_parameter("vt", [D, SK], BF16, isOutput=False)
    wq_d = nc.declare_dram_parameter("wq", [D, DL], BF16, isOutput=False)
    # wkg = W_K @ blockdiag(W_gen_S) folded on the host, local columns only
    wkg_d = nc.declare_dram_parameter("wkg", [D, DL], BF16, isOutput=False)
    wv_d = nc.declare_dram_parameter("wv", [D, DL], BF16, isOutput=False)
    wm_d = nc.declare_dram_parameter("wm", [DL, D], BF16, isOutput=False)
    out_d = nc.declare_dram_parameter("out", [SQF, D], F32, isOutput=True)

    with tile.TileContext(nc) as tc:
        with tc.tile_pool(name="cst", bufs=1) as cst, \
             tc.tile_pool(name="pt", bufs=8) as ptp, \
             tc.tile_pool(name="dve", bufs=2) as dvp, \
             tc.tile_pool(name="psS", bufs=2, space="PSUM") as psS, \
             tc.tile_pool(name="psV", bufs=2, space="PSUM") as psV:

            # ---- loads (128-partition layout), ordered so group 0's
            # operands land first
            wq = cst.tile([P, ND, DL], BF16, tag="wq")
            nc.sync.dma_start(wq[:], wq_d.rearrange("(k p) e -> p k e", p=P))
            qt = cst.tile([P, ND, SQF], BF16, tag="qt")
            qt_r = qt_d.rearrange("(k p) q -> p k q", p=P)
            nc.sync.dma_start(qt[:, :, 0:SQ], qt_r[:, :, 0:SQ])
            wkg = cst.tile([P, ND, DL], BF16, tag="wkg")
            nc.sync.dma_start(wkg[:], wkg_d.rearrange("(k p) e -> p k e", p=P))
            kt = cst.tile([P, ND, SK], BF16, tag="kt")
            kt_r = kt_d.rearrange("(k p) s -> p k s", p=P)
            nc.sync.dma_start(kt[:, :, 0:512], kt_r[:, :, 0:512])
            nc.sync.dma_start(kt[:, :, 512:1024], kt_r[:, :, 512:1024])
            wv = cst.tile([P, ND, DL], BF16, tag="wv")
            nc.sync.dma_start(wv[:], wv_d.rearrange("(k p) e -> p k e", p=P))
            vt = cst.tile([P, ND, SK], BF16, tag="vt")
            vt_r = vt_d.rearrange("(k p) s -> p k s", p=P)
            # vt in quarters so vproj(0..) can start early
            nc.sync.dma_start(vt[:, :, 0:512], vt_r[:, :, 0:512])
            nc.sync.dma_start(qt[:, :, SQ:SQF], qt_r[:, :, SQ:SQF])
            nc.sync.dma_start(kt[:, :, SK // 2:], kt_r[:, :, SK // 2:])
            nc.sync.dma_start(vt[:, :, 512:1024], vt_r[:, :, 512:1024])
            nc.sync.dma_start(vt[:, :, 1024:SK], vt_r[:, :, 1024:SK])
            wm = cst.tile([P, NDL, D], BF16, tag="wm")
            nc.sync.dma_start(wm[:], wm_d.rearrange("(j p) o -> p j o", p=P))

            # selector for the rowsum-reciprocal partition broadcast with a
            # K=33 contraction: row 0 -> output partitions 0:64 (head a),
            # row 32 -> partitions 64:128 (head b); partition bases must be
            # 32-aligned, hence rows 0/32
            sel = cst.tile([33, P], BF16, tag="sel")
            nc.vector.memset(sel[:], 0.0)
            nc.vector.memset(sel[0:1, 0:DH], 1.0)
            nc.vector.memset(sel[32:33, DH:P], 1.0)

            # ---- projections (emitted as chunks, interleaved below) ----
            qlt = [cst.tile([P, SQF], BF16, tag=f"qlt{j}", name=f"qlt{j}")
                   for j in range(NJ)]
            vaug = [cst.tile([P, HL, DH + 1], BF16, tag=f"vaug{i}",
                             name=f"vaug{i}") for i in range(NKB)]
            khwt = [cst.tile([P, SK], BF16, tag=f"khwt{j}", name=f"khwt{j}")
                    for j in range(NJ)]

            def qproj_h(j, half):
                # Q_l^T rows = local e in [128j, 128j+128), q half
                ps = psS.tile([P, 1024], F32, tag="psS", name="psq")
                for k in range(ND):
                    for qc in range(2):
                        sc = half * 1024 + qc * 512
                        nc.tensor.matmul(
                            ps[:, qc * 512:(qc + 1) * 512],
                            wq[:, k, j * P:(j + 1) * P],
                            qt[:, k, sc:sc + 512],
                            start=(k == 0), stop=(k == ND - 1))
                nc.vector.tensor_copy(
                    out=qlt[j][:, half * 1024:(half + 1) * 1024], in_=ps[:])

            def kproj(j, half):
                # K_hw^T (W_gen_S folded): rows 0:64 = local head 2j,
                # rows 64:128 = local head 2j+1
                ps = psS.tile([P, 1024], F32, tag="psS", name="psk")
                for k in range(ND):
                    for qc in range(2):
                        sc = half * 1024 + qc * 512
                        nc.tensor.matmul(
                            ps[:, qc * 512:(qc + 1) * 512],
                            wkg[:, k, j * P:(j + 1) * P],
                            kt[:, k, sc:sc + 512],
                            start=(k == 0), stop=(k == ND - 1))
                nc.vector.tensor_copy(
                    out=khwt[j][:, half * 1024:(half + 1) * 1024], in_=ps[:])

            def kproj_q(j, sc):
                # quarter-granular kproj so early scores can start sooner
                ps = psS.tile([P, 1024], F32, tag="psS", name="pskq")
                for k in range(ND):
                    nc.tensor.matmul(
                        ps[:, 0:512],
                        wkg[:, k, j * P:(j + 1) * P],
                        kt[:, k, sc:sc + 512],
                        start=(k == 0), stop=(k == ND - 1))
                nc.vector.tensor_copy(
                    out=khwt[j][:, sc:sc + 512], in_=ps[:, 0:512])

            def vproj(i):
                # V_l rows [128i, 128i+128), local heads + the ones column
                ps = psS.tile([P, 1024], F32, tag="psS", name="psv")
                for k in range(ND):
                    nc.tensor.matmul(
                        ps[:, 0:DL],
                        vt[:, k, i * P:(i + 1) * P],
                        wv[:, k, :],
                        start=(k == 0), stop=(k == ND - 1))
                nc.vector.tensor_copy(
                    out=vaug[i][:, :, 0:DH],
                    in_=ps[:, 0:DL].rearrange("p (h v) -> p h v", v=DH))
                nc.vector.memset(vaug[i][:, :, DH:DH + 1], 1.0)

            def outproj(qb, headt=None):
                # partial out-proj for query block qb: both local pairs
                # accumulated, then straight to DRAM (host sums core pairs)
                qh = qb // 8
                ps = psS.tile([P, 1024], F32, tag="psS", name="pop")
                for pr in range(NJ):
                    g = pr * 2 + qh
                    nc.tensor.matmul(ps[:, 0:512],
                                     headt[g][:, (qb % 8) * P:(qb % 8 + 1) * P],
                                     wm[:, pr, :], start=(pr == 0),
                                     stop=(pr == NJ - 1))
                ot = dvp.tile([P, D], F32, tag="ot", name="ot")
                nc.vector.tensor_copy(out=ot[:], in_=ps[:, 0:512])
                nc.sync.dma_start(out_d[qb * P:(qb + 1) * P, :], ot[:])

            # ---- attention: one continuous software-pipelined stream over
            # all (group, kblock) steps, group = (pair, qhalf).
            headt = [cst.tile([P, SQ], BF16, tag=f"headt{g}", name=f"headt{g}")
                     for g in range(NG)]
            LOGF = mybir.ActivationFunctionType.Ln
            LAG = 4
            SS = NG * NKB
            pv_tiles = {}
            pts = {}

            def scores_step(pos):
                g, t = divmod(pos, NKB)
                pr, qh = divmod(g, 2)
                q0 = qh * 1024
                psa = psS.tile([P, SQ], F32, tag="psS", name="psa")
                for qc in range(2):
                    s = qc * 512
                    nc.tensor.matmul(
                        psa[:, s:s + 512],
                        khwt[pr][0:DH, t * P:(t + 1) * P],
                        qlt[pr][0:DH, q0 + s:q0 + s + 512],
                        start=True, stop=True)
                pta = ptp.tile([P, SQ], BF16, tag="pt", name="pta")
                nc.scalar.activation(pta[:], psa[:], EXPF)
                psb = psS.tile([P, SQ], F32, tag="psS", name="psb")
                for qc in range(2):
                    s = qc * 512
                    nc.tensor.matmul(
                        psb[:, s:s + 512],
                        khwt[pr][DH:P, t * P:(t + 1) * P],
                        qlt[pr][DH:P, q0 + s:q0 + s + 512],
                        start=True, stop=True, tile_position=(DH, 0))
                # head b's exp runs on DVE (bit-trick), halving ScalarE load
                ptb = ptp.tile([P, SQ], I16, tag="pti", name="ptb")
                nc.vector.tensor_scalar(ptb[:], psb[:], _EXPC1, _EXPC2,
                                        MULT, mybir.AluOpType.add)
                pts[pos] = (pta, ptb)

            def pv_step(pos):
                g, t = divmod(pos, NKB)
                pr = g // 2
                if t == 0:
                    pv_tiles[g] = (
                        psV.tile([DH + 1, SQ], F32, tag="psV", name="pva"),
                        psV.tile([DH + 1, SQ], F32, tag="psV", name="pvb"))
                pva, pvb = pv_tiles[g]
                pta, ptb = pts.pop(pos)
                st, sp = (t == 0), (t == NKB - 1)
                # group per stationary so each vaug head is loaded once
                for qc in range(2):
                    s = qc * 512
                    nc.tensor.matmul(pva[:, s:s + 512],
                                     vaug[t][:, 2 * pr, :],
                                     pta[:, s:s + 512], start=st, stop=sp)
                for qc in range(2):
                    s = qc * 512
                    nc.tensor.matmul(pvb[:, s:s + 512],
                                     vaug[t][:, 2 * pr + 1, :],
                                     ptb[:, s:s + 512].bitcast(BF16),
                                     start=st, stop=sp)

            def part1(g):
                # 1/x = exp(-ln(x)) on ScalarE, reading the rowsum rows
                # STRAIGHT from psum (a single-partition DVE gather costs
                # ~1.2us because it only uses one lane - avoid it entirely)
                pva, pvb = pv_tiles[g]
                lg = dvp.tile([33, SQ], F32, tag="lg", name="lg")
                nc.vector.memset(lg[:], 1.0)
                nc.scalar.activation(lg[0:1, :], pva[DH:DH + 1, :], LOGF)
                nc.scalar.activation(lg[32:33, :], pvb[DH:DH + 1, :], LOGF)
                recr = dvp.tile([33, SQ], BF16, tag="recr", name="recr")
                nc.scalar.activation(recr[:], lg[:], EXPF, scale=-1.0)
                return recr

            recrs = {}

            def part2(g):
                pva, pvb = pv_tiles.pop(g)
                recr = recrs.pop(g)
                rbp = psS.tile([P, SQ], F32, tag="psS", name="rbp")
                for qc in range(2):
                    s = qc * 512
                    nc.tensor.matmul(rbp[:, s:s + 512], sel[:],
                                     recr[:, s:s + 512], start=True, stop=True)
                rbe = dvp.tile([DH, SQ], F32, tag="rbe", name="rbe")
                rbo = dvp.tile([DH, SQ], F32, tag="rbo", name="rbo")
                nc.vector.tensor_copy(out=rbe[:], in_=rbp[0:DH, :])
                nc.vector.tensor_copy(out=rbo[:], in_=rbp[DH:P, :])
                nc.vector.tensor_tensor(headt[g][0:DH, :], pva[0:DH, :],
                                        rbe[:], MULT)
                nc.vector.tensor_tensor(headt[g][DH:P, :], pvb[0:DH, :],
                                        rbo[:], MULT)

            # stream schedule: extra chunks keyed by scores position
            pre = {}

            def at(pos, f):
                pre.setdefault(pos, []).append(f)

            for i in range(NKB):
                at(i, lambda i=i: vproj(i))
            # pair 0 remainder (prologue covers qlt0-h0 and khwt0 cols 0:1024)
            at(2, lambda: qproj_h(0, 1))
            at(4, lambda: kproj_q(0, 1024))
            at(6, lambda: kproj_q(0, 1536))
            # pair 1 (needed from pos 32), placed after the group-boundary
            # congestion (pv taper + part1/part2 around pos 17-19)
            at(20, lambda: qproj_h(1, 0))
            at(22, lambda: qproj_h(1, 1))
            at(24, lambda: kproj_q(1, 0))
            at(26, lambda: kproj_q(1, 512))
            at(28, lambda: kproj_q(1, 1024))
            at(30, lambda: kproj_q(1, 1536))
            # part2 one step early (PV lag tapers on each group's last step),
            # so the next group's PV does not stall on the psV slot handoff
            for g in range(NG - 1):
                at(NKB * (g + 1) + 3, lambda g=g: part2(g))
            # qhalf-0 output blocks (need part2 of groups 0 and 2)
            for qb in range(8):
                at(53 + 2 * qb if qb < 6 else 59 + qb, lambda qb=qb: outproj(qb, headt))

            # prologue: warm the PE clock-gate with dummy matmuls while the
            # first DMAs land, then emit what group 0's scores need
            warm = cst.tile([P, 512], BF16, tag="warm")
            nc.vector.memset(warm[:], 0.0)
            for _ in range(3):
                wps = psS.tile([P, 1024], F32, tag="psS", name="wps")
                for r in range(12):
                    nc.tensor.matmul(wps[:, (r % 2) * 512:(r % 2) * 512 + 512],
                                     warm[:, 0:P], warm[:],
                                     start=True, stop=True)
            qproj_h(0, 0)
            kproj(0, 0)

            pv_at = {}
            for p_ in range(SS):
                # taper the lag on each group's last pv step so part1/part2
                # can run before the next group needs the psV slots (only the
                # last step tapers -- more bunches too many MMs on one step)
                tl = p_ % NKB
                lag = 3 if tl == 15 else LAG
                pv_at.setdefault(p_ + lag, []).append(p_)
            for pos in range(SS + LAG):
                if pos < SS:
                    scores_step(pos)
                for f in pre.get(pos, []):
                    f()
                for p_ in pv_at.get(pos, []):
                    pv_step(p_)
                    pg, pt_ = divmod(p_, NKB)
                    if pt_ == NKB - 1:
                        recrs[pg] = part1(pg)

            # ---- tail: group 3 (pair1, qhalf1) normalize + the qhalf-1
            # output blocks, processed per q-half so the chain pipelines
            gl = NG - 1
            # keep the PE clock-gate warm through the part1 latency gap --
            # otherwise HAM re-throttles and the tail matmuls run at 1.2 GHz
            wps2 = psS.tile([P, 1024], F32, tag="psS", name="wps2")
            for r in range(10):
                nc.tensor.matmul(wps2[:, (r % 2) * 512:(r % 2) * 512 + 512],
                                 warm[:, 0:P], warm[:], start=True, stop=True)
            pvs_a, pvs_b = pv_tiles.pop(gl)
            recr = recrs.pop(gl)
            for qhalf in range(2):
                s = qhalf * 512
                rbp = psS.tile([P, 1024], F32, tag="psS", name="rbp")
                nc.tensor.matmul(rbp[:, 0:512], sel[:], recr[:, s:s + 512],
                                 start=True, stop=True)
                # no EXPs remain here, so ScalarE is free: put the copies on
                # it and keep only the normalize multiplies on the DVE, which
                # would otherwise serialize the whole tail
                rbe = dvp.tile([DH, 512], F32, tag="rbeh", name="rbeh")
                rbo = dvp.tile([DH, 512], F32, tag="rboh", name="rboh")
                nc.scalar.copy(out=rbe[:], in_=rbp[0:DH, 0:512])
                nc.scalar.copy(out=rbo[:], in_=rbp[DH:P, 0:512])
                nc.vector.tensor_tensor(headt[gl][0:DH, s:s + 512],
                                        pvs_a[0:DH, s:s + 512], rbe[:], MULT)
                nc.vector.tensor_tensor(headt[gl][DH:P, s:s + 512],
                                        pvs_b[0:DH, s:s + 512], rbo[:], MULT)
                for qb in range(8 + qhalf * 4, 8 + qhalf * 4 + 4):
                    ps = psS.tile([P, 1024], F32, tag="psS")
                    for pr in range(NJ):
                        g = pr * 2 + 1
                        nc.tensor.matmul(
                            ps[:, 0:512],
                            headt[g][:, (qb % 8) * P:(qb % 8 + 1) * P],
                            wm[:, pr, :], start=(pr == 0), stop=(pr == NJ - 1))
                    ot = dvp.tile([P, D], F32, tag="ot", name="ot")
                    if qb % 2 == 0:
                        nc.scalar.copy(out=ot[:], in_=ps[:, 0:512])
                    else:
                        nc.vector.tensor_copy(out=ot[:], in_=ps[:, 0:512])
                    nc.sync.dma_start(out_d[qb * P:(qb + 1) * P, :], ot[:])

    _split_multi_waits(nc)
    return nc


_NC = None


def _get_nc():
    global _NC
    if _NC is None:
        _NC = _build_nc()
    return _NC


def _prep_in_maps(Q, K, V, W_Q, W_K, W_V, W_gen_S, W_multi_head):
    bf = ml_dtypes.bfloat16
    wq_f = np.asarray(W_Q, np.float32)
    wv_f = np.asarray(W_V, np.float32)
    wm_f = np.asarray(W_multi_head, np.float32)
    # fold W_gen_S into W_K: K_hw = K @ W_K @ blockdiag(W_gen_S)
    wk_f = np.asarray(W_K, np.float32)
    wg_f = np.asarray(W_gen_S, np.float32)
    wkg_f = np.einsum('dhe,ef->dhf', wk_f.reshape(D, H, DH), wg_f)
    wkg_f = wkg_f.reshape(D, D)

    Q = np.asarray(Q, np.float32)
    K = np.asarray(K, np.float32)
    V = np.asarray(V, np.float32)

    qts = [np.ascontiguousarray(Q[b].T).astype(bf) for b in range(4)]
    kts = [np.ascontiguousarray(K[b].T).astype(bf) for b in range(4)]
    vts = [np.ascontiguousarray(V[b].T).astype(bf) for b in range(4)]

    in_maps = []
    for c in range(8):
        b, hp = divmod(c, 2)
        sl = slice(hp * DL, (hp + 1) * DL)
        in_maps.append({
            "qt": qts[b], "kt": kts[b], "vt": vts[b],
            "wq": np.ascontiguousarray(wq_f[:, sl]).astype(bf),
            "wkg": np.ascontiguousarray(wkg_f[:, sl]).astype(bf),
            "wv": np.ascontiguousarray(wv_f[:, sl]).astype(bf),
            "wm": np.ascontiguousarray(wm_f[sl, :]).astype(bf),
        })
    return in_maps


def _run(in_maps, trace=False):
    nc = _get_nc()
    res = run_bass_kernel_spmd(nc, in_maps, list(range(8)), trace=trace)
    out = np.empty((4, SQF, D), np.float32)
    for b in range(4):
        out[b] = res.results[2 * b]["out"] + res.results[2 * b + 1]["out"]
    return out, res


def kernel(Q, K, V, M, W_Q, W_K, W_V, W_gen_S, W_multi_head):
    in_maps = _prep_in_maps(Q, K, V, W_Q, W_K, W_V, W_gen_S, W_multi_head)
    out, _ = _run(in_maps, trace=False)
    return out


def kernel_traced(Q, K, V, M, W_Q, W_K, W_V, W_gen_S, W_multi_head):
    in_maps = _prep_in_maps(Q, K, V, W_Q, W_K, W_V, W_gen_S, W_multi_head)
    return _run(in_maps, trace=True)


# revision 65
# speedup vs baseline: 1.0080x; 1.0080x over previous
"""Multi-head attention ('general' similarity, softmax, out-proj) on 8 trn2
NeuronCores via Bass/Tile.

Sharding (head-split): core c handles batch b=c//2 and head group hp=c%2
(heads 4hp..4hp+4 = 2 pairs), over the FULL query range.  Each core projects
K/V only for its 4 heads (halving the projection work vs a q-split), computes
its heads' attention for all 2048 queries, and emits a PARTIAL output
projection; the host sums the two partials per batch.

Device layout trick: everything is kept feature-major ("transposed") so every
matmul contraction runs along SBUF partitions:
  Q_l^T[e,q]   = sum_d W_Q[d,e] Q^T[d,q]          (host supplies Q^T)
  K_hw^T[e,s]  = sum_d Wg[d,e] K_l^T[d,s]         (W_gen_S folded into W_K)
  scores^T[k,q]= sum_e K_hw^T[e,k] Q_l^T[e,q]     (row-packed: 2 heads share PE)
  P^T          = exp(scores^T)                     (head a: ScalarE exp; head b:
                 one DVE tensor_scalar emitting bf16 exp bits into int16)
  headaug^T    = sum_k Vaug[k,(v,1)] P^T[k,q]     (65th 'ones' col => rowsums free)
  out[q,o]    += sum_hv (head^T/rowsum)[hv,q] W_m[hv,o]   (partial, host-summed)

The 64 pipeline steps are indexed by g=(pair,qhalf) in 0..3 and kblock t in
0..15 -- the same shape as the previous q-split kernel, so the software
pipeline (LAG, part1/part2 positions) carries over unchanged.
"""
import sys
import types

import numpy as np
import ml_dtypes

# ---------------------------------------------------------------- axon shim --
# antenv in this image lacks axon_hooks; register the NTFF profiling hook
# ourselves so trace=True works when the caller asks for it.
def _ensure_axon_hooks():
    if 'antenv.axon_hooks' in sys.modules:
        return
    try:
        from trn_agent_boot.trn_boot import _ntff_profile_via_ctypes
        hook = _ntff_profile_via_ctypes('/opt/axon/libaxon_pjrt.so')
    except Exception:
        hook = None
    mod = types.ModuleType('antenv.axon_hooks')
    mod.get_axon_ntff_profile_hook = lambda: hook
    mod.set_axon_ntff_profile_hook = lambda h: None
    sys.modules['antenv.axon_hooks'] = mod


_ensure_axon_hooks()

import concourse.bass as bass
import concourse.mybir as mybir
import concourse.tile as tile
from concourse.bass_utils import run_bass_kernel_spmd

BF16 = mybir.dt.bfloat16
F32 = mybir.dt.float32
I16 = mybir.dt.int16

# one-instruction exp on DVE: bf16 bits of exp(s) ~= int16(round(s*128/ln2
# + (127*128 - 5.5))).  Rel err ~N(+1%, 1.8%); the +1% common-mode bias
# cancels in softmax normalization.  Verified bit-exact vs hardware model.
_EXPC1 = 128.0 / 0.6931471805599453
_EXPC2 = 16256.0 - 5.5

P = 128
D = 512          # model dim (= D_K = D_V = D_OUT)
H = 8            # total heads (host-side fold uses this)
SQF = 2048       # full query range per core
SQ = 1024        # query rows per pipeline group (half range)
SK = 2048        # key rows (full sequence)
HL = 4           # local heads per core
DH = 64
DL = HL * DH     # local feature width (256)
NJ = HL // 2     # local head pairs (2)
NG = 4           # pipeline groups: (pair, qhalf)
NKB = SK // P    # 16 key blocks
NQB = SQF // P   # 16 query blocks (full range)
ND = D // P      # 4 feature blocks
NDL = DL // P    # 2 local-feature blocks
EXPF = mybir.ActivationFunctionType.Exp
MULT = mybir.AluOpType.mult


# ------------------------------------------------------- walrus workaround --
# This container's walrus accepts only ONE embedded sync-wait per hw
# instruction. Move all but the last wait of any instruction onto single-wait
# NoOps inserted just before it in the same engine stream.
_SPLIT_CTR = [0]


def _split_multi_waits(nc, max_waits=1):
    def mk_nop(engine, wait):
        _SPLIT_CTR[0] += 1
        nop = mybir.InstNoOp(name=f"antsplitw-{_SPLIT_CTR[0]}", ins=[], outs=[])
        nop.engine = engine
        nop.sync_info = mybir.SyncInfo(on_wait=[wait], on_update=[])
        return nop

    for fn in nc.m.functions:
        for bb in fn.blocks:
            out = []
            changed = False
            for inst in bb.instructions:
                si = inst.sync_info
                waits = list(si.on_wait) if si is not None and si.on_wait else []
                if len(waits) > max_waits:
                    for w in waits[:-max_waits]:
                        out.append(mk_nop(inst.engine, w))
                    si.on_wait = waits[-max_waits:]
                    changed = True
                out.append(inst)
            if changed:
                bb.instructions = out


# ------------------------------------------------------------ device kernel --
def _build_nc():
    nc = bass.Bass("TRN2", target_bir_lowering=False, debug=False)

    qt_d = nc.declare_dram_parameter("qt", [D, SQF], BF16, isOutput=False)
    kt_d = nc.declare_dram_parameter("kt", [D, SK], BF16, isOutput=False)
    vt_d = nc.declare_dram_parameter("vt", [D, SK], BF16, isOutput=False)
    wq_d = nc.declare_dram_parameter("wq", [D, DL], BF16, isOutput=False)
    # wkg = W_K @ blockdiag(W_gen_S) folded on the host, local columns only
    wkg_d = nc.declare_dram_parameter("wkg", [D, DL], BF16, isOutput=False)
    wv_d = nc.declare_dram_parameter("wv", [D, DL], BF16, isOutput=False)
    wm_d = nc.declare_dram_parameter("wm", [DL, D], BF16, isOutput=False)
    out_d = nc.declare_dram_parameter("out", [SQF, D], F32, isOutput=True)

    with tile.TileContext(nc) as tc:
        with tc.tile_pool(name="cst", bufs=1) as cst, \
             tc.tile_pool(name="pt", bufs=8) as ptp, \
             tc.tile_pool(name="dve", bufs=2) as dvp, \
             tc.tile_pool(name="psS", bufs=2, space="PSUM") as psS, \
             tc.tile_pool(name="psV", bufs=2, space="PSUM") as psV:

            # ---- loads (128-partition layout), ordered so group 0's
            # operands land first
            wq = cst.tile([P, ND, DL], BF16, tag="wq")
            nc.sync.dma_start(wq[:], wq_d.rearrange("(k p) e -> p k e", p=P))
            qt = cst.tile([P, ND, SQF], BF16, tag="qt")
            qt_r = qt_d.rearrange("(k p) q -> p k q", p=P)
            nc.sync.dma_start(qt[:, :, 0:SQ], qt_r[:, :, 0:SQ])
            wkg = cst.tile([P, ND, DL], BF16, tag="wkg")
            nc.sync.dma_start(wkg[:], wkg_d.rearrange("(k p) e -> p k e", p=P))
            kt = cst.tile([P, ND, SK], BF16, tag="kt")
            kt_r = kt_d.rearrange("(k p) s -> p k s", p=P)
            nc.sync.dma_start(kt[:, :, 0:SK // 2], kt_r[:, :, 0:SK // 2])
            wv = cst.tile([P, ND, DL], BF16, tag="wv")
            nc.sync.dma_start(wv[:], wv_d.rearrange("(k p) e -> p k e", p=P))
            vt = cst.tile([P, ND, SK], BF16, tag="vt")
            vt_r = vt_d.rearrange("(k p) s -> p k s", p=P)
            # vt in quarters so vproj(0..) can start early
            nc.sync.dma_start(vt[:, :, 0:512], vt_r[:, :, 0:512])
            nc.sync.dma_start(qt[:, :, SQ:SQF], qt_r[:, :, SQ:SQF])
            nc.sync.dma_start(kt[:, :, SK // 2:], kt_r[:, :, SK // 2:])
            nc.sync.dma_start(vt[:, :, 512:1024], vt_r[:, :, 512:1024])
            nc.sync.dma_start(vt[:, :, 1024:SK], vt_r[:, :, 1024:SK])
            wm = cst.tile([P, NDL, D], BF16, tag="wm")
            nc.sync.dma_start(wm[:], wm_d.rearrange("(j p) o -> p j o", p=P))

            # selector for the rowsum-reciprocal partition broadcast with a
            # K=33 contraction: row 0 -> output partitions 0:64 (head a),
            # row 32 -> partitions 64:128 (head b); partition bases must be
            # 32-aligned, hence rows 0/32
            sel = cst.tile([33, P], BF16, tag="sel")
            nc.vector.memset(sel[:], 0.0)
            nc.vector.memset(sel[0:1, 0:DH], 1.0)
            nc.vector.memset(sel[32:33, DH:P], 1.0)

            # ---- projections (emitted as chunks, interleaved below) ----
            qlt = [cst.tile([P, SQF], BF16, tag=f"qlt{j}", name=f"qlt{j}")
                   for j in range(NJ)]
            vaug = [cst.tile([P, HL, DH + 1], BF16, tag=f"vaug{i}",
                             name=f"vaug{i}") for i in range(NKB)]
            khwt = [cst.tile([P, SK], BF16, tag=f"khwt{j}", name=f"khwt{j}")
                    for j in range(NJ)]

            def qproj_h(j, half):
                # Q_l^T rows = local e in [128j, 128j+128), q half
                ps = psS.tile([P, 1024], F32, tag="psS", name="psq")
                for k in range(ND):
                    for qc in range(2):
                        sc = half * 1024 + qc * 512
                        nc.tensor.matmul(
                            ps[:, qc * 512:(qc + 1) * 512],
                            wq[:, k, j * P:(j + 1) * P],
                            qt[:, k, sc:sc + 512],
                            start=(k == 0), stop=(k == ND - 1))
                nc.vector.tensor_copy(
                    out=qlt[j][:, half * 1024:(half + 1) * 1024], in_=ps[:])

            def kproj(j, half):
                # K_hw^T (W_gen_S folded): rows 0:64 = local head 2j,
                # rows 64:128 = local head 2j+1
                ps = psS.tile([P, 1024], F32, tag="psS", name="psk")
                for k in range(ND):
                    for qc in range(2):
                        sc = half * 1024 + qc * 512
                        nc.tensor.matmul(
                            ps[:, qc * 512:(qc + 1) * 512],
                            wkg[:, k, j * P:(j + 1) * P],
                            kt[:, k, sc:sc + 512],
                            start=(k == 0), stop=(k == ND - 1))
                nc.vector.tensor_copy(
                    out=khwt[j][:, half * 1024:(half + 1) * 1024], in_=ps[:])

            def kproj_q(j, sc):
                # quarter-granular kproj so early scores can start sooner
                ps = psS.tile([P, 1024], F32, tag="psS", name="pskq")
                for k in range(ND):
                    nc.tensor.matmul(
                        ps[:, 0:512],
                        wkg[:, k, j * P:(j + 1) * P],
                        kt[:, k, sc:sc + 512],
                        start=(k == 0), stop=(k == ND - 1))
                nc.vector.tensor_copy(
                    out=khwt[j][:, sc:sc + 512], in_=ps[:, 0:512])

            def vproj(i):
                # V_l rows [128i, 128i+128), local heads + the ones column
                ps = psS.tile([P, 1024], F32, tag="psS", name="psv")
                for k in range(ND):
                    nc.tensor.matmul(
                        ps[:, 0:DL],
                        vt[:, k, i * P:(i + 1) * P],
                        wv[:, k, :],
                        start=(k == 0), stop=(k == ND - 1))
                nc.vector.tensor_copy(
                    out=vaug[i][:, :, 0:DH],
                    in_=ps[:, 0:DL].rearrange("p (h v) -> p h v", v=DH))
                nc.vector.memset(vaug[i][:, :, DH:DH + 1], 1.0)

            def outproj(qb, headt=None):
                # partial out-proj for query block qb: both local pairs
                # accumulated, then straight to DRAM (host sums core pairs)
                qh = qb // 8
                ps = psS.tile([P, 1024], F32, tag="psS", name="pop")
                for pr in range(NJ):
                    g = pr * 2 + qh
                    nc.tensor.matmul(ps[:, 0:512],
                                     headt[g][:, (qb % 8) * P:(qb % 8 + 1) * P],
                                     wm[:, pr, :], start=(pr == 0),
                                     stop=(pr == NJ - 1))
                ot = dvp.tile([P, D], F32, tag="ot", name="ot")
                nc.vector.tensor_copy(out=ot[:], in_=ps[:, 0:512])
                nc.sync.dma_start(out_d[qb * P:(qb + 1) * P, :], ot[:])

            # ---- attention: one continuous software-pipelined stream over
            # all (group, kblock) steps, group = (pair, qhalf).
            headt = [cst.tile([P, SQ], BF16, tag=f"headt{g}", name=f"headt{g}")
                     for g in range(NG)]
            LOGF = mybir.ActivationFunctionType.Ln
            LAG = 4
            SS = NG * NKB
            pv_tiles = {}
            pts = {}

            def scores_step(pos):
                g, t = divmod(pos, NKB)
                pr, qh = divmod(g, 2)
                q0 = qh * 1024
                psa = psS.tile([P, SQ], F32, tag="psS", name="psa")
                for qc in range(2):
                    s = qc * 512
                    nc.tensor.matmul(
                        psa[:, s:s + 512],
                        khwt[pr][0:DH, t * P:(t + 1) * P],
                        qlt[pr][0:DH, q0 + s:q0 + s + 512],
                        start=True, stop=True)
                pta = ptp.tile([P, SQ], BF16, tag="pt", name="pta")
                nc.scalar.activation(pta[:], psa[:], EXPF)
                psb = psS.tile([P, SQ], F32, tag="psS", name="psb")
                for qc in range(2):
                    s = qc * 512
                    nc.tensor.matmul(
                        psb[:, s:s + 512],
                        khwt[pr][DH:P, t * P:(t + 1) * P],
                        qlt[pr][DH:P, q0 + s:q0 + s + 512],
                        start=True, stop=True, tile_position=(DH, 0))
                # head b's exp runs on DVE (bit-trick), halving ScalarE load
                ptb = ptp.tile([P, SQ], I16, tag="pti", name="ptb")
                nc.vector.tensor_scalar(ptb[:], psb[:], _EXPC1, _EXPC2,
                                        MULT, mybir.AluOpType.add)
                pts[pos] = (pta, ptb)

            def pv_step(pos):
                g, t = divmod(pos, NKB)
                pr = g // 2
                if t == 0:
                    pv_tiles[g] = (
                        psV.tile([DH + 1, SQ], F32, tag="psV", name="pva"),
                        psV.tile([DH + 1, SQ], F32, tag="psV", name="pvb"))
                pva, pvb = pv_tiles[g]
                pta, ptb = pts.pop(pos)
                st, sp = (t == 0), (t == NKB - 1)
                # group per stationary so each vaug head is loaded once
                for qc in range(2):
                    s = qc * 512
                    nc.tensor.matmul(pva[:, s:s + 512],
                                     vaug[t][:, 2 * pr, :],
                                     pta[:, s:s + 512], start=st, stop=sp)
                for qc in range(2):
                    s = qc * 512
                    nc.tensor.matmul(pvb[:, s:s + 512],
                                     vaug[t][:, 2 * pr + 1, :],
                                     ptb[:, s:s + 512].bitcast(BF16),
                                     start=st, stop=sp)

            def part1(g):
                # 1/x = exp(-ln(x)) on ScalarE, reading the rowsum rows
                # STRAIGHT from psum (a single-partition DVE gather costs
                # ~1.2us because it only uses one lane - avoid it entirely)
                pva, pvb = pv_tiles[g]
                lg = dvp.tile([33, SQ], F32, tag="lg", name="lg")
                nc.vector.memset(lg[:], 1.0)
                nc.scalar.activation(lg[0:1, :], pva[DH:DH + 1, :], LOGF)
                nc.scalar.activation(lg[32:33, :], pvb[DH:DH + 1, :], LOGF)
                recr = dvp.tile([33, SQ], BF16, tag="recr", name="recr")
                nc.scalar.activation(recr[:], lg[:], EXPF, scale=-1.0)
                return recr

            recrs = {}

            def part2(g):
                pva, pvb = pv_tiles.pop(g)
                recr = recrs.pop(g)
                rbp = psS.tile([P, SQ], F32, tag="psS", name="rbp")
                for qc in range(2):
                    s = qc * 512
                    nc.tensor.matmul(rbp[:, s:s + 512], sel[:],
                                     recr[:, s:s + 512], start=True, stop=True)
                rbe = dvp.tile([DH, SQ], F32, tag="rbe", name="rbe")
                rbo = dvp.tile([DH, SQ], F32, tag="rbo", name="rbo")
                nc.vector.tensor_copy(out=rbe[:], in_=rbp[0:DH, :])
                nc.vector.tensor_copy(out=rbo[:], in_=rbp[DH:P, :])
                nc.vector.tensor_tensor(headt[g][0:DH, :], pva[0:DH, :],
                                        rbe[:], MULT)
                nc.vector.tensor_tensor(headt[g][DH:P, :], pvb[0:DH, :],
                                        rbo[:], MULT)

            # stream schedule: extra chunks keyed by scores position
            pre = {}

            def at(pos, f):
                pre.setdefault(pos, []).append(f)

            for i in range(NKB):
                at(i, lambda i=i: vproj(i))
            # pair 0 remainder (prologue covers qlt0-h0 and khwt0 cols 0:1024)
            at(2, lambda: qproj_h(0, 1))
            at(4, lambda: kproj_q(0, 1024))
            at(6, lambda: kproj_q(0, 1536))
            # pair 1 (needed from pos 32), placed after the group-boundary
            # congestion (pv taper + part1/part2 around pos 17-19)
            at(20, lambda: qproj_h(1, 0))
            at(22, lambda: qproj_h(1, 1))
            at(24, lambda: kproj_q(1, 0))
            at(26, lambda: kproj_q(1, 512))
            at(28, lambda: kproj_q(1, 1024))
            at(30, lambda: kproj_q(1, 1536))
            # part2 one step early (PV lag tapers on each group's last step),
            # so the next group's PV does not stall on the psV slot handoff
            for g in range(NG - 1):
                at(NKB * (g + 1) + 3, lambda g=g: part2(g))
            # qhalf-0 output blocks (need part2 of groups 0 and 2)
            for qb in range(8):
                at(53 + 2 * qb if qb < 6 else 59 + qb, lambda qb=qb: outproj(qb, headt))

            # prologue: warm the PE clock-gate with dummy matmuls while the
            # first DMAs land, then emit what group 0's scores need
            warm = cst.tile([P, 512], BF16, tag="warm")
            nc.vector.memset(warm[:], 0.0)
            for _ in range(3):
                wps = psS.tile([P, 1024], F32, tag="psS", name="wps")
                for r in range(12):
                    nc.tensor.matmul(wps[:, (r % 2) * 512:(r % 2) * 512 + 512],
                                     warm[:, 0:P], warm[:],
                                     start=True, stop=True)
            qproj_h(0, 0)
            kproj(0, 0)

            pv_at = {}
            for p_ in range(SS):
                # taper the lag on each group's last pv step so part1/part2
                # can run before the next group needs the psV slots (only the
                # last step tapers -- more bunches too many MMs on one step)
                tl = p_ % NKB
                lag = 3 if tl == 15 else LAG
                pv_at.setdefault(p_ + lag, []).append(p_)
            for pos in range(SS + LAG):
                if pos < SS:
                    scores_step(pos)
                for f in pre.get(pos, []):
                    f()
                for p_ in pv_at.get(pos, []):
                    pv_step(p_)
                    pg, pt_ = divmod(p_, NKB)
                    if pt_ == NKB - 1:
                        recrs[pg] = part1(pg)

            # ---- tail: group 3 (pair1, qhalf1) normalize + the qhalf-1
            # output blocks, processed per q-half so the chain pipelines
            gl = NG - 1
            # keep the PE clock-gate warm through the part1 latency gap --
            # otherwise HAM re-throttles and the tail matmuls run at 1.2 GHz
            wps2 = psS.tile([P, 1024], F32, tag="psS", name="wps2")
            for r in range(10):
                nc.tensor.matmul(wps2[:, (r % 2) * 512:(r % 2) * 512 + 512],
                                 warm[:, 0:P], warm[:], start=True, stop=True)
            pvs_a, pvs_b = pv_tiles.pop(gl)
            recr = recrs.pop(gl)
            for qhalf in range(2):
                s = qhalf * 512
                rbp = psS.tile([P, 1024], F32, tag="psS", name="rbp")
                nc.tensor.matmul(rbp[:, 0:512], sel[:], recr[:, s:s + 512],
                                 start=True, stop=True)
                # no EXPs remain here, so ScalarE is free: put the copies on
                # it and keep only the normalize multiplies on the DVE, which
                # would otherwise serialize the whole tail
                rbe = dvp.tile([DH, 512], F32, tag="rbeh", name="rbeh")
                rbo = dvp.tile([DH, 512], F32, tag="rboh", name="rboh")
                nc.scalar.copy(out=rbe[:], in_=rbp[0:DH, 0:512])
                nc.scalar.copy(out=rbo[:], in_=rbp[DH:P, 0:512])
                nc.vector.tensor_tensor(headt[gl][0:DH, s:s + 512],
                                        pvs_a[0:DH, s:s + 512], rbe[:], MULT)
                nc.vector.tensor_tensor(headt[gl][DH:P, s:s + 512],
                                        pvs_b[0:DH, s:s + 512], rbo[:], MULT)
                for qb in range(8 + qhalf * 4, 8 + qhalf * 4 + 4):
                    ps = psS.tile([P, 1024], F32, tag="psS")
                    for pr in range(NJ):
                        g = pr * 2 + 1
                        nc.tensor.matmul(
                            ps[:, 0:512],
                            headt[g][:, (qb % 8) * P:(qb % 8 + 1) * P],
                            wm[:, pr, :], start=(pr == 0), stop=(pr == NJ - 1))
                    ot = dvp.tile([P, D], F32, tag="ot", name="ot")
                    if qb % 2 == 0:
                        nc.scalar.copy(out=ot[:], in_=ps[:, 0:512])
                    else:
                        nc.vector.tensor_copy(out=ot[:], in_=ps[:, 0:512])
                    nc.sync.dma_start(out_d[qb * P:(qb + 1) * P, :], ot[:])

    _split_multi_waits(nc)
    return nc


_NC = None


def _get_nc():
    global _NC
    if _NC is None:
        _NC = _build_nc()
    return _NC


def _prep_in_maps(Q, K, V, W_Q, W_K, W_V, W_gen_S, W_multi_head):
    bf = ml_dtypes.bfloat16
    wq_f = np.asarray(W_Q, np.float32)
    wv_f = np.asarray(W_V, np.float32)
    wm_f = np.asarray(W_multi_head, np.float32)
    # fold W_gen_S into W_K: K_hw = K @ W_K @ blockdiag(W_gen_S)
    wk_f = np.asarray(W_K, np.float32)
    wg_f = np.asarray(W_gen_S, np.float32)
    wkg_f = np.einsum('dhe,ef->dhf', wk_f.reshape(D, H, DH), wg_f)
    wkg_f = wkg_f.reshape(D, D)

    Q = np.asarray(Q, np.float32)
    K = np.asarray(K, np.float32)
    V = np.asarray(V, np.float32)

    qts = [np.ascontiguousarray(Q[b].T).astype(bf) for b in range(4)]
    kts = [np.ascontiguousarray(K[b].T).astype(bf) for b in range(4)]
    vts = [np.ascontiguousarray(V[b].T).astype(bf) for b in range(4)]

    in_maps = []
    for c in range(8):
        b, hp = divmod(c, 2)
        sl = slice(hp * DL, (hp + 1) * DL)
        in_maps.append({
            "qt": qts[b], "kt": kts[b], "vt": vts[b],
            "wq": np.ascontiguousarray(wq_f[:, sl]).astype(bf),
            "wkg": np.ascontiguousarray(wkg_f[:, sl]).astype(bf),
            "wv": np.ascontiguousarray(wv_f[:, sl]).astype(bf),
            "wm": np.ascontiguousarray(wm_f[sl, :]).astype(bf),
        })
    return in_maps


def _run(in_maps, trace=False):
    nc = _get_nc()
    res = run_bass_kernel_spmd(nc, in_maps, list(range(8)), trace=trace)
    out = np.empty((4, SQF, D), np.float32)
    for b in range(4):
        out[b] = res.results[2 * b]["out"] + res.results[2 * b + 1]["out"]
    return out, res


def kernel(Q, K, V, M, W_Q, W_K, W_V, W_gen_S, W_multi_head):
    in_maps = _prep_in_maps(Q, K, V, W_Q, W_K, W_V, W_gen_S, W_multi_head)
    out, _ = _run(in_maps, trace=False)
    return out


def kernel_traced(Q, K, V, M, W_Q, W_K, W_V, W_gen_S, W_multi_head):
    in_maps = _prep_in_maps(Q, K, V, W_Q, W_K, W_V, W_gen_S, W_multi_head)
    return _run(in_maps, trace=True)
